# revision 2
# baseline (speedup 1.0000x reference)
"""Multi-head attention (B=2, S=2048, D=768, H=12) on 8 NeuronCores.

Sharding: data-parallel over batch (2) x tensor-parallel over heads (4 groups
of 3 heads) = 8 cores. Each core computes its 3 heads' Q/K/V projections,
attention, and a partial output projection; the host sums the 4 per-batch
partials and adds the output bias.

Per-core kernel layout:
  xT   [768, 2048]  input transposed (d on partitions, 6 chunks of 128), f32r
  QT,KT[192, 2048]  transposed projections in bf16 (head-major rows, bias via
                    rank-1 ones matmul, f32 PSUM accumulation)
  V    [2048, 3x65] natural-layout bf16 V with a ones column appended per
                    head: the ctx matmul lhsT [sk, 65] then yields softmax
                    denominators in PSUM row 64 for free.
  Attention loops sq-chunk (4x512) outermost, then head, then sk-chunk:
    scoresT [sk 128, sq 512] per (c,h,i) in PSUM -> Exp on ScalarE
    (scale=1/sqrt(dk) folded) -> bf16 SBUF -> ctx matmul accumulates
    [65, 512] in PSUM over i. Normalized on eviction via
    reciprocal_approx_fast + partition_broadcast. Output projection and
    output DMA run per sq-chunk so store traffic overlaps compute.
  The V projection runs immediately after the Q/K projection matmuls so the
  tensor engine has dense work across the proj->attention transition (a PE
  idle gap >3.4us there re-throttles the PE clock to half rate for the rest
  of the kernel).
"""

import sys

sys.path.insert(0, "/opt/trn_rl_repo")

import numpy as np

B, S, D = 2, 2048, 768
H, DK = 12, 64
P = 128
HG = 3              # heads per core
E = HG * DK         # 192: per-core projection width
KD = D // P         # 6 contraction chunks
SQC = S // 512      # 4 sq chunks of 512
SKC = S // P        # 16 sk chunks of 128
SCALE = 1.0 / 8.0   # 1/sqrt(DK)

_NC_CACHE = {}


def _build_bass(body_reps=1):
    import concourse.bacc as bacc
    import concourse.tile as tile
    from concourse import mybir

    f32 = mybir.dt.float32
    f32r = mybir.dt.float32r
    bf16 = mybir.dt.bfloat16
    Exp = mybir.ActivationFunctionType.Exp

    nc = bacc.Bacc(trn_type="TRN2", debug=False)

    xT = nc.dram_tensor("xT", [D, S], f32, kind="ExternalInput")
    wqT = nc.dram_tensor("wqT", [D, E], f32, kind="ExternalInput")
    wkT = nc.dram_tensor("wkT", [D, E], f32, kind="ExternalInput")
    wvT = nc.dram_tensor("wvT", [D, 256], f32, kind="ExternalInput")
    bq = nc.dram_tensor("bq", [1, E], f32, kind="ExternalInput")
    bk = nc.dram_tensor("bk", [1, E], f32, kind="ExternalInput")
    bv = nc.dram_tensor("bv", [1, 256], f32, kind="ExternalInput")
    woT = nc.dram_tensor("woT", [E, D], f32, kind="ExternalInput")
    ones_d = nc.dram_tensor("ones", [P, 512], f32, kind="ExternalInput")
    outT = nc.dram_tensor("outT", [D, S], f32, kind="ExternalOutput")

    xT_d = xT.ap().rearrange("(c p) s -> c p s", p=P)
    wqT_d = wqT.ap().rearrange("(c p) e -> c p e", p=P)
    wkT_d = wkT.ap().rearrange("(c p) e -> c p e", p=P)
    wvT_d = wvT.ap().rearrange("(c p) e -> c p e", p=P)
    outT_d = outT.ap().rearrange("(c p) s -> c p s", p=P)

    with tile.TileContext(nc) as tc:
        for _rep in range(body_reps):
            with tc.tile_pool(name="persist", bufs=1) as persist, \
                 tc.tile_pool(name="work", bufs=4) as work, \
                 tc.tile_pool(name="small", bufs=2) as small:

                # ---- load inputs (f32r via dtype-punned DMA: PE truncates) ----
                x_sb = []
                for d in range(KD):
                    t = persist.tile([P, S], f32r, tag=f"x{d}")
                    nc.sync.dma_start(out=t[:], in_=xT_d[d].bitcast(f32r))
                    x_sb.append(t)
                wq_sb, wk_sb, wv_sb = [], [], []
                for d in range(KD):
                    t = persist.tile([P, E], f32r, tag=f"wq{d}")
                    nc.sync.dma_start(out=t[:], in_=wqT_d[d].bitcast(f32r))
                    wq_sb.append(t)
                    t = persist.tile([P, E], f32r, tag=f"wk{d}")
                    nc.sync.dma_start(out=t[:], in_=wkT_d[d].bitcast(f32r))
                    wk_sb.append(t)
                    t = persist.tile([P, 256], f32r, tag=f"wv{d}")
                    nc.sync.dma_start(out=t[:], in_=wvT_d[d].bitcast(f32r))
                    wv_sb.append(t)
                bq_sb = persist.tile([1, E], f32r, tag="bq")
                nc.sync.dma_start(out=bq_sb[:], in_=bq.ap().bitcast(f32r))
                bk_sb = persist.tile([1, E], f32r, tag="bk")
                nc.sync.dma_start(out=bk_sb[:], in_=bk.ap().bitcast(f32r))
                bv_sb = persist.tile([1, 256], f32r, tag="bv")
                nc.sync.dma_start(out=bv_sb[:], in_=bv.ap().bitcast(f32r))
                wo_a = persist.tile([P, D], f32r, tag="wo_a")
                nc.sync.dma_start(out=wo_a[:], in_=woT.ap()[0:P, :].bitcast(f32r))
                wo_b = persist.tile([64, D], f32r, tag="wo_b")
                nc.sync.dma_start(out=wo_b[:], in_=woT.ap()[P:E, :].bitcast(f32r))

                ones = persist.tile([P, 512], f32r, tag="ones")
                nc.sync.dma_start(out=ones[:], in_=ones_d.ap().bitcast(f32r))

                # preload the exp activation table while the PE is still
                # chewing on projections (first ACTIVATE to a new table set
                # costs ~2.7us)
                et_warm = small.tile([1, 8], f32, tag="warm")
                nc.scalar.activation(et_warm[:], bq_sb[0:1, 0:8], Exp, scale=1.0)

                # ---- persistent activations ----
                qt_a = persist.tile([P, S], bf16, tag="qt_a")   # heads 0,1
                qt_b = persist.tile([64, S], bf16, tag="qt_b")  # head 2
                kt_a = persist.tile([P, S], bf16, tag="kt_a")
                kt_b = persist.tile([64, S], bf16, tag="kt_b")
                v_sb = [persist.tile([P, HG, 65], bf16, tag=f"v{i}", name=f"v{i}") for i in range(SKC)]
                ctx_a = persist.tile([P, S], f32r, tag="ctx_a")
                ctx_b = persist.tile([64, S], f32r, tag="ctx_b")

                # ================= QKV projections =================
                with tc.tile_pool(name="proj_ps", bufs=8, space="PSUM") as proj_ps:
                    for (w_chunks, b_tile, dst_a, dst_b) in (
                        (wq_sb, bq_sb, qt_a, qt_b),
                        (wk_sb, bk_sb, kt_a, kt_b),
                    ):
                        ps = []
                        for m in range(2):  # e-tiles: [0:128], [128:192]
                            mw = P if m == 0 else 64
                            for c in range(SQC):
                                ps.append(proj_ps.tile([mw, 512], f32, tag="proj", name=f"proj_ps_{m}_{c}"))
                        for d in range(KD):
                            k = 0
                            for m in range(2):
                                mw = P if m == 0 else 64
                                for c in range(SQC):
                                    nc.tensor.matmul(
                                        ps[k][:],
                                        w_chunks[d][:, m * P : m * P + mw],
                                        x_sb[d][:, c * 512 : (c + 1) * 512],
                                        start=(d == 0), stop=False,
                                    )
                                    k += 1
                        k = 0
                        for m in range(2):
                            mw = P if m == 0 else 64
                            for c in range(SQC):
                                nc.tensor.matmul(
                                    ps[k][:],
                                    b_tile[0:1, m * P : m * P + mw],
                                    ones[0:1, 0:512],
                                    start=False, stop=True,
                                )
                                k += 1
                        k = 0
                        for m in range(2):
                            mw = P if m == 0 else 64
                            dst = dst_a if m == 0 else dst_b
                            for c in range(SQC):
                                nc.vector.tensor_copy(
                                    dst[0:mw, c * 512 : (c + 1) * 512], ps[k][:]
                                )
                                k += 1

                    # ---- V projection: dense PE work bridging into attention
                    for i in range(SKC):
                        vps = proj_ps.tile([P, 256], f32, tag="proj", name=f"vps_{i}")
                        for d in range(KD):
                            nc.tensor.matmul(
                                vps[:],
                                x_sb[d][:, i * P : (i + 1) * P],
                                wv_sb[d][:],
                                start=(d == 0), stop=False,
                            )
                        nc.tensor.matmul(
                            vps[:], ones[0:1, 0:P], bv_sb[0:1, :],
                            start=False, stop=True,
                        )
                        nc.vector.tensor_copy(
                            v_sb[i][:, :, 64:65], ones[:, 0:3][:, :, None]
                        )
                        nc.vector.tensor_copy(
                            v_sb[i][:, :, 0:64],
                            vps[:, 0:E].rearrange("p (h d) -> p h d", h=HG),
                        )

                # ================= attention (sq-chunk outermost) =============
                with tc.tile_pool(name="sc_ps", bufs=3, space="PSUM") as sc_ps, \
                     tc.tile_pool(name="ctx_ps", bufs=3, space="PSUM") as ctx_ps, \
                     tc.tile_pool(name="op_ps", bufs=2, space="PSUM") as op_ps:
                    for c in range(SQC):
                        for h in range(HG):
                            if h < 2:
                                kt_h = kt_a[h * 64 : (h + 1) * 64, :]
                                qt_h = qt_a[h * 64 : (h + 1) * 64, :]
                                ctx_h = ctx_a[h * 64 : (h + 1) * 64, :]
                            else:
                                kt_h = kt_b[0:64, :]
                                qt_h = qt_b[0:64, :]
                                ctx_h = ctx_b[0:64, :]

                            cps = ctx_ps.tile([65, 512], f32, tag="ctx", name=f"cps_{c}_{h}")
                            for i in range(SKC):
                                sp = sc_ps.tile([P, 512], f32, tag="sc", name=f"sp_{c}_{h}_{i}")
                                nc.tensor.matmul(
                                    sp[:],
                                    kt_h[:, i * P : (i + 1) * P],
                                    qt_h[:, c * 512 : (c + 1) * 512],
                                    start=True, stop=True,
                                )
                                et = work.tile([P, 512], bf16, tag="exp", name=f"et_{c}_{h}_{i}")
                                nc.scalar.activation(et[:], sp[:], Exp, scale=SCALE)
                                nc.tensor.matmul(
                                    cps[:],
                                    v_sb[i][:, h, :],
                                    et[:],
                                    start=(i == 0), stop=(i == SKC - 1),
                                )
                            # normalize: row 64 of cps holds the denominator
                            den = small.tile([1, 512], f32, tag="den")
                            nc.vector.tensor_copy(den[:], cps[64:65, :])
                            r = small.tile([1, 512], f32, tag="r")
                            nc.vector.reciprocal_approx_fast(r[:], den[:])
                            rb = small.tile([64, 512], f32, tag="rb")
                            nc.gpsimd.partition_broadcast(rb[:], r[:])
                            nc.vector.tensor_mul(
                                ctx_h[:, c * 512 : (c + 1) * 512],
                                cps[0:64, :],
                                rb[:],
                            )
                        # output projection for this sq chunk (all heads done)
                        for e in range(KD):
                            op = op_ps.tile([P, 512], f32, tag="op", name=f"op_{c}_{e}")
                            nc.tensor.matmul(
                                op[:],
                                wo_a[:, e * P : (e + 1) * P],
                                ctx_a[:, c * 512 : (c + 1) * 512],
                                start=True, stop=False,
                            )
                            nc.tensor.matmul(
                                op[:],
                                wo_b[:, e * P : (e + 1) * P],
                                ctx_b[:, c * 512 : (c + 1) * 512],
                                start=False, stop=True,
                            )
                            o = work.tile([P, 512], f32, tag="o", bufs=6)
                            if e % 2 == 0:
                                nc.vector.tensor_copy(o[:], op[:])
                            else:
                                nc.scalar.activation(
                                    o[:], op[:],
                                    mybir.ActivationFunctionType.Copy,
                                )
                            nc.sync.dma_start(
                                out=outT_d[e][:, c * 512 : (c + 1) * 512], in_=o[:]
                            )

    nc.finalize()
    return nc


def _get_nc(body_reps=1):
    key = ("nc", body_reps)
    if key not in _NC_CACHE:
        _NC_CACHE[key] = _build_bass(body_reps)
    return _NC_CACHE[key]


def _core_inputs(c, x, w_q, b_q, w_k, b_k, w_v, b_v, w_o):
    b, g = divmod(c, 4)
    gs = slice(g * E, (g + 1) * E)
    wv_pad = np.zeros((D, 256), np.float32)
    wv_pad[:, :E] = np.ascontiguousarray(w_v[gs, :].T)
    bv_pad = np.zeros((1, 256), np.float32)
    bv_pad[0, :E] = b_v[gs]
    return {
        "xT": np.ascontiguousarray(x[b].T),
        "wqT": np.ascontiguousarray(w_q[gs, :].T),
        "wkT": np.ascontiguousarray(w_k[gs, :].T),
        "wvT": wv_pad,
        "bq": b_q[gs].reshape(1, E).astype(np.float32),
        "bk": b_k[gs].reshape(1, E).astype(np.float32),
        "bv": bv_pad,
        "woT": np.ascontiguousarray(w_o[:, gs].T),
        "ones": np.ones((P, 512), np.float32),
    }


def kernel(x, w_q, b_q, w_k, b_k, w_v, b_v, w_o, b_o, _trace=False):
    from concourse.bass_utils import run_bass_kernel_spmd

    x = np.asarray(x, np.float32)
    args = [np.asarray(a, np.float32) for a in
            (w_q, b_q, w_k, b_k, w_v, b_v, w_o)]
    b_o = np.asarray(b_o, np.float32)

    nc = _get_nc()
    in_maps = [_core_inputs(c, x, *args) for c in range(8)]
    res = run_bass_kernel_spmd(nc, in_maps, core_ids=list(range(8)), trace=_trace)

    out = np.zeros((B, S, D), np.float32)
    for c in range(8):
        out[c // 4] += res.results[c]["outT"].T
    out += b_o
    if _trace:
        kernel._last_results = res
    return out


# revision 3
# speedup vs baseline: 1.0870x; 1.0870x over previous
"""Multi-head attention (B=2, S=2048, D=768, H=12) on 8 NeuronCores.

Sharding: data-parallel over batch (2) x tensor-parallel over heads (4 groups
of 3 heads) = 8 cores. Each core computes its 3 heads' Q/K/V projections,
attention, and a partial output projection; the host sums the 4 per-batch
partials and adds the output bias.

Per-core kernel layout (all matmuls in float32r: 1 cycle/row at N>=256):
  xT   [768, 2048]  input transposed (d on partitions, 6 chunks of 128)
  QT,KT[192, 2048]  transposed projections (head-major rows, bias via
                    rank-1 ones matmul)
  V    [2048, 3x65] natural-layout V with a ones column appended per head:
                    the ctx matmul lhsT [sk, 65] then yields softmax
                    denominators in PSUM row 64 for free.
  scoresT [sk 128, sq] per (head, sk-chunk) in PSUM -> Exp on ScalarE
                    (scale=1/sqrt(dk) folded into the activation) -> SBUF
  ctxT accumulates over sk in PSUM [65, 512] per sq-chunk; normalized on
                    eviction via reciprocal_approx_fast + partition_broadcast
  outT [768, 2048] partial output projection, host-summed across head groups

Perf notes (vs the first working version):
  - DMAs are emitted weights-first and x interleaved per contraction chunk so
    the first projection matmuls start ~4us in instead of after all loads.
  - The V projection runs as a dense PE block right after the Q/K projection
    matmuls: a PE idle gap >3.4us at the proj->attention boundary re-throttles
    the PE clock to half rate (HAM) for the rest of the kernel.
  - The exp activation table is preloaded during the projection phase.
"""

import sys

sys.path.insert(0, "/opt/trn_rl_repo")

import numpy as np

B, S, D = 2, 2048, 768
H, DK = 12, 64
P = 128
HG = 3              # heads per core
E = HG * DK         # 192: per-core projection width
KD = D // P         # 6 contraction chunks
SQC = S // 512      # 4 sq chunks of 512
SKC = S // P        # 16 sk chunks of 128
SCALE = 1.0 / 8.0   # 1/sqrt(DK)

_NC_CACHE = {}


def _build_bass(body_reps=1):
    import concourse.bacc as bacc
    import concourse.tile as tile
    from concourse import mybir

    f32 = mybir.dt.float32
    f32r = mybir.dt.float32r
    Exp = mybir.ActivationFunctionType.Exp

    nc = bacc.Bacc(trn_type="TRN2", debug=False)

    xT = nc.dram_tensor("xT", [D, S], f32, kind="ExternalInput")
    wqT = nc.dram_tensor("wqT", [D, E], f32, kind="ExternalInput")
    wkT = nc.dram_tensor("wkT", [D, E], f32, kind="ExternalInput")
    wvT = nc.dram_tensor("wvT", [D, 256], f32, kind="ExternalInput")
    bq = nc.dram_tensor("bq", [1, E], f32, kind="ExternalInput")
    bk = nc.dram_tensor("bk", [1, E], f32, kind="ExternalInput")
    bv = nc.dram_tensor("bv", [1, 256], f32, kind="ExternalInput")
    woT = nc.dram_tensor("woT", [E, D], f32, kind="ExternalInput")
    ones_d = nc.dram_tensor("ones", [P, 512], f32, kind="ExternalInput")
    outT = nc.dram_tensor("outT", [D, S], f32, kind="ExternalOutput")

    xT_d = xT.ap().rearrange("(c p) s -> c p s", p=P)
    wqT_d = wqT.ap().rearrange("(c p) e -> c p e", p=P)
    wkT_d = wkT.ap().rearrange("(c p) e -> c p e", p=P)
    wvT_d = wvT.ap().rearrange("(c p) e -> c p e", p=P)
    outT_d = outT.ap().rearrange("(c p) s -> c p s", p=P)

    with tile.TileContext(nc) as tc:
        for _rep in range(body_reps):
            with tc.tile_pool(name="persist", bufs=1) as persist, \
                 tc.tile_pool(name="work", bufs=4) as work, \
                 tc.tile_pool(name="small", bufs=2) as small:

                # ---- load inputs (f32r via dtype-punned DMA: PE truncates).
                # Emission order = DMA queue order: per-chunk weights first,
                # then that chunk of x, so projection matmuls for chunk d can
                # start as soon as its inputs land.
                bq_sb = persist.tile([1, E], f32r, tag="bq")
                nc.sync.dma_start(out=bq_sb[:], in_=bq.ap().bitcast(f32r))
                bk_sb = persist.tile([1, E], f32r, tag="bk")
                nc.sync.dma_start(out=bk_sb[:], in_=bk.ap().bitcast(f32r))
                bv_sb = persist.tile([1, 256], f32r, tag="bv")
                nc.sync.dma_start(out=bv_sb[:], in_=bv.ap().bitcast(f32r))
                ones = persist.tile([P, 512], f32r, tag="ones")
                nc.sync.dma_start(out=ones[:], in_=ones_d.ap().bitcast(f32r))

                x_sb, wq_sb, wk_sb, wv_sb = [], [], [], []
                for d in range(KD):
                    t = persist.tile([P, E], f32r, tag=f"wq{d}")
                    nc.sync.dma_start(out=t[:], in_=wqT_d[d].bitcast(f32r))
                    wq_sb.append(t)
                    t = persist.tile([P, E], f32r, tag=f"wk{d}")
                    nc.sync.dma_start(out=t[:], in_=wkT_d[d].bitcast(f32r))
                    wk_sb.append(t)
                    t = persist.tile([P, S], f32r, tag=f"x{d}")
                    nc.sync.dma_start(out=t[:], in_=xT_d[d].bitcast(f32r))
                    x_sb.append(t)
                for d in range(KD):
                    t = persist.tile([P, 256], f32r, tag=f"wv{d}")
                    nc.sync.dma_start(out=t[:], in_=wvT_d[d].bitcast(f32r))
                    wv_sb.append(t)
                wo_a = persist.tile([P, D], f32r, tag="wo_a")
                nc.sync.dma_start(out=wo_a[:], in_=woT.ap()[0:P, :].bitcast(f32r))
                wo_b = persist.tile([64, D], f32r, tag="wo_b")
                nc.sync.dma_start(out=wo_b[:], in_=woT.ap()[P:E, :].bitcast(f32r))

                # preload the exp activation table while the PE chews on
                # projections (first ACTIVATE to a new table set costs ~2.7us)
                et_warm = small.tile([1, 8], f32, tag="warm")
                nc.scalar.activation(et_warm[:], bq_sb[0:1, 0:8], Exp, scale=1.0)

                # ---- persistent activations ----
                qt_a = persist.tile([P, S], f32r, tag="qt_a")   # heads 0,1
                qt_b = persist.tile([64, S], f32r, tag="qt_b")  # head 2
                kt_a = persist.tile([P, S], f32r, tag="kt_a")
                kt_b = persist.tile([64, S], f32r, tag="kt_b")
                v_sb = [persist.tile([P, HG, 65], f32r, tag=f"v{i}", name=f"v{i}") for i in range(SKC)]
                ctx_a = persist.tile([P, S], f32r, tag="ctx_a")
                ctx_b = persist.tile([64, S], f32r, tag="ctx_b")

                # ================= QKV projections =================
                with tc.tile_pool(name="proj_ps", bufs=8, space="PSUM") as proj_ps:
                    for (w_chunks, b_tile, dst_a, dst_b) in (
                        (wq_sb, bq_sb, qt_a, qt_b),
                        (wk_sb, bk_sb, kt_a, kt_b),
                    ):
                        ps = []
                        for m in range(2):  # e-tiles: [0:128], [128:192]
                            mw = P if m == 0 else 64
                            for c in range(SQC):
                                ps.append(proj_ps.tile([mw, 512], f32, tag="proj", name=f"proj_ps_{m}_{c}"))
                        for d in range(KD):
                            k = 0
                            for m in range(2):
                                mw = P if m == 0 else 64
                                for c in range(SQC):
                                    nc.tensor.matmul(
                                        ps[k][:],
                                        w_chunks[d][:, m * P : m * P + mw],
                                        x_sb[d][:, c * 512 : (c + 1) * 512],
                                        start=(d == 0), stop=False,
                                    )
                                    k += 1
                        k = 0
                        for m in range(2):
                            mw = P if m == 0 else 64
                            for c in range(SQC):
                                nc.tensor.matmul(
                                    ps[k][:],
                                    b_tile[0:1, m * P : m * P + mw],
                                    ones[0:1, 0:512],
                                    start=False, stop=True,
                                )
                                k += 1
                        k = 0
                        for m in range(2):
                            mw = P if m == 0 else 64
                            dst = dst_a if m == 0 else dst_b
                            for c in range(SQC):
                                nc.vector.tensor_copy(
                                    dst[0:mw, c * 512 : (c + 1) * 512], ps[k][:]
                                )
                                k += 1

                    # ---- V projection: dense PE work bridging into attention
                    for i in range(SKC):
                        vps = proj_ps.tile([P, 256], f32, tag="proj", name=f"vps_{i}")
                        for d in range(KD):
                            nc.tensor.matmul(
                                vps[:],
                                x_sb[d][:, i * P : (i + 1) * P],
                                wv_sb[d][:],
                                start=(d == 0), stop=False,
                            )
                        nc.tensor.matmul(
                            vps[:], ones[0:1, 0:P], bv_sb[0:1, :],
                            start=False, stop=True,
                        )
                        nc.vector.tensor_copy(
                            v_sb[i][:, :, 64:65], ones[:, 0:3][:, :, None]
                        )
                        nc.vector.tensor_copy(
                            v_sb[i][:, :, 0:64],
                            vps[:, 0:E].rearrange("p (h d) -> p h d", h=HG),
                        )

                # ================= attention =================
                with tc.tile_pool(name="sc_ps", bufs=2, space="PSUM") as sc_ps, \
                     tc.tile_pool(name="ctx_ps", bufs=4, space="PSUM") as ctx_ps:
                    for h in range(HG):
                        if h < 2:
                            kt_h = kt_a[h * 64 : (h + 1) * 64, :]
                            qt_h = qt_a[h * 64 : (h + 1) * 64, :]
                            ctx_h = ctx_a[h * 64 : (h + 1) * 64, :]
                        else:
                            kt_h = kt_b[0:64, :]
                            qt_h = qt_b[0:64, :]
                            ctx_h = ctx_b[0:64, :]

                        cps = [ctx_ps.tile([65, 512], f32, tag="ctx", name=f"cps_{h}_{c}") for c in range(SQC)]
                        for i in range(SKC):
                            sps, ets = [], []
                            for half in range(2):
                                sp = sc_ps.tile([P, 1024], f32, tag="sc", name=f"sp_{h}_{i}_{half}")
                                for j in range(2):
                                    nc.tensor.matmul(
                                        sp[:, j * 512 : (j + 1) * 512],
                                        kt_h[:, i * P : (i + 1) * P],
                                        qt_h[:, half * 1024 + j * 512 : half * 1024 + (j + 1) * 512],
                                        start=True, stop=True,
                                    )
                                sps.append(sp)
                            for half in range(2):
                                et = work.tile([P, 1024], f32r, tag="exp", name=f"et_{h}_{i}_{half}")
                                nc.scalar.activation(et[:], sps[half][:], Exp, scale=SCALE)
                                ets.append(et)
                            for half in range(2):
                                for j in range(2):
                                    c = half * 2 + j
                                    nc.tensor.matmul(
                                        cps[c][:],
                                        v_sb[i][:, h, :],
                                        ets[half][:, j * 512 : (j + 1) * 512],
                                        start=(i == 0), stop=(i == SKC - 1),
                                    )
                        for c in range(SQC):
                            den = small.tile([1, 512], f32, tag="den")
                            nc.vector.tensor_copy(den[:], cps[c][64:65, :])
                            r = small.tile([1, 512], f32, tag="r")
                            nc.vector.reciprocal_approx_fast(r[:], den[:])
                            rb = small.tile([64, 512], f32, tag="rb")
                            nc.gpsimd.partition_broadcast(rb[:], r[:])
                            nc.vector.tensor_mul(
                                ctx_h[:, c * 512 : (c + 1) * 512],
                                cps[c][0:64, :],
                                rb[:],
                            )
                            if h == HG - 1:
                                # output projection for this sq chunk (all heads done)
                                for e in range(KD):
                                    op = ctx_ps.tile([P, 512], f32, tag="ctx",
                                                     name=f"op_{e}_{c}")
                                    nc.tensor.matmul(
                                        op[:],
                                        wo_a[:, e * P : (e + 1) * P],
                                        ctx_a[:, c * 512 : (c + 1) * 512],
                                        start=True, stop=False,
                                    )
                                    nc.tensor.matmul(
                                        op[:],
                                        wo_b[:, e * P : (e + 1) * P],
                                        ctx_b[:, c * 512 : (c + 1) * 512],
                                        start=False, stop=True,
                                    )
                                    o = work.tile([P, 512], f32, tag="o", bufs=6)
                                    if e % 2 == 0:
                                        nc.vector.tensor_copy(o[:], op[:])
                                    else:
                                        nc.scalar.activation(
                                            o[:], op[:],
                                            mybir.ActivationFunctionType.Copy,
                                        )
                                    nc.sync.dma_start(
                                        out=outT_d[e][:, c * 512 : (c + 1) * 512], in_=o[:]
                                    )

    nc.finalize()
    return nc


def _get_nc(body_reps=1):
    key = ("nc", body_reps)
    if key not in _NC_CACHE:
        _NC_CACHE[key] = _build_bass(body_reps)
    return _NC_CACHE[key]


def _core_inputs(c, x, w_q, b_q, w_k, b_k, w_v, b_v, w_o):
    b, g = divmod(c, 4)
    gs = slice(g * E, (g + 1) * E)
    wv_pad = np.zeros((D, 256), np.float32)
    wv_pad[:, :E] = np.ascontiguousarray(w_v[gs, :].T)
    bv_pad = np.zeros((1, 256), np.float32)
    bv_pad[0, :E] = b_v[gs]
    return {
        "xT": np.ascontiguousarray(x[b].T),
        "wqT": np.ascontiguousarray(w_q[gs, :].T),
        "wkT": np.ascontiguousarray(w_k[gs, :].T),
        "wvT": wv_pad,
        "bq": b_q[gs].reshape(1, E).astype(np.float32),
        "bk": b_k[gs].reshape(1, E).astype(np.float32),
        "bv": bv_pad,
        "woT": np.ascontiguousarray(w_o[:, gs].T),
        "ones": np.ones((P, 512), np.float32),
    }


def kernel(x, w_q, b_q, w_k, b_k, w_v, b_v, w_o, b_o, _trace=False):
    from concourse.bass_utils import run_bass_kernel_spmd

    x = np.asarray(x, np.float32)
    args = [np.asarray(a, np.float32) for a in
            (w_q, b_q, w_k, b_k, w_v, b_v, w_o)]
    b_o = np.asarray(b_o, np.float32)

    nc = _get_nc()
    in_maps = [_core_inputs(c, x, *args) for c in range(8)]
    res = run_bass_kernel_spmd(nc, in_maps, core_ids=list(range(8)), trace=_trace)

    out = np.zeros((B, S, D), np.float32)
    for c in range(8):
        out[c // 4] += res.results[c]["outT"].T
    out += b_o
    if _trace:
        kernel._last_results = res
    return out


# revision 9
# speedup vs baseline: 1.1580x; 1.0653x over previous
"""Multi-head attention (B=2, S=2048, D=768, H=12) on 8 NeuronCores.

Sharding: data-parallel over batch (2) x tensor-parallel over heads (4 groups
of 3 heads) = 8 cores. Each core computes its 3 heads' Q/K/V projections,
attention, and a partial output projection; the host sums the 4 per-batch
partials and adds the output bias.

Per-core kernel layout (all matmuls in float32r: 1 cycle/row at N>=256):
  xT   [768, 2048]  input transposed (d on partitions, 6 chunks of 128)
  QT,KT[192, 2048]  transposed projections (head-major rows, bias via
                    rank-1 ones matmul)
  V    [2048, 3x65] natural-layout V with a ones column appended per head:
                    the ctx matmul lhsT [sk, 65] then yields softmax
                    denominators in PSUM row 64 for free.
  scoresT [sk 128, sq] per (head, sk-chunk) in PSUM -> Exp on ScalarE
                    (scale=1/sqrt(dk) folded into the activation) -> SBUF
  ctxT accumulates over sk in PSUM [65, 512] per sq-chunk; normalized on
                    eviction via reciprocal_approx_fast + partition_broadcast
  outT [768, 2048] partial output projection, host-summed across head groups

Perf notes (vs the first working version):
  - DMAs are emitted weights-first and x interleaved per contraction chunk so
    the first projection matmuls start ~4us in instead of after all loads.
  - The V projection runs as a dense PE block right after the Q/K projection
    matmuls: a PE idle gap >3.4us at the proj->attention boundary re-throttles
    the PE clock to half rate (HAM) for the rest of the kernel.
  - The exp activation table is preloaded during the projection phase.
"""

import sys

sys.path.insert(0, "/opt/trn_rl_repo")

import numpy as np

B, S, D = 2, 2048, 768
H, DK = 12, 64
P = 128
HG = 3              # heads per core
E = HG * DK         # 192: per-core projection width
KD = D // P         # 6 contraction chunks
SQC = S // 512      # 4 sq chunks of 512
SKC = S // P        # 16 sk chunks of 128
SCALE = 1.0 / 8.0   # 1/sqrt(DK)

_NC_CACHE = {}


def _build_bass(body_reps=1):
    import concourse.bacc as bacc
    import concourse.tile as tile
    from concourse import mybir

    f32 = mybir.dt.float32
    f32r = mybir.dt.float32r
    Exp = mybir.ActivationFunctionType.Exp

    nc = bacc.Bacc(trn_type="TRN2", debug=False)

    xT = nc.dram_tensor("xT", [D, S], f32, kind="ExternalInput")
    wqT = nc.dram_tensor("wqT", [D, E], f32, kind="ExternalInput")
    wkT = nc.dram_tensor("wkT", [D, E], f32, kind="ExternalInput")
    wvT = nc.dram_tensor("wvT", [D, 256], f32, kind="ExternalInput")
    bq = nc.dram_tensor("bq", [1, E], f32, kind="ExternalInput")
    bk = nc.dram_tensor("bk", [1, E], f32, kind="ExternalInput")
    bv = nc.dram_tensor("bv", [1, 256], f32, kind="ExternalInput")
    woT = nc.dram_tensor("woT", [E, D], f32, kind="ExternalInput")
    ones_d = nc.dram_tensor("ones", [P, 512], f32, kind="ExternalInput")
    outT = nc.dram_tensor("outT", [D, S], f32, kind="ExternalOutput")

    xT_d = xT.ap().rearrange("(c p) s -> c p s", p=P)
    wqT_d = wqT.ap().rearrange("(c p) e -> c p e", p=P)
    wkT_d = wkT.ap().rearrange("(c p) e -> c p e", p=P)
    wvT_d = wvT.ap().rearrange("(c p) e -> c p e", p=P)
    outT_d = outT.ap().rearrange("(c p) s -> c p s", p=P)

    with tile.TileContext(nc) as tc:
        for _rep in range(body_reps):
            with tc.tile_pool(name="persist", bufs=1) as persist, \
                 tc.tile_pool(name="work", bufs=4) as work, \
                 tc.tile_pool(name="small", bufs=2) as small:

                # ---- load inputs (f32r via dtype-punned DMA: PE truncates).
                # Emission order = DMA queue order: per-chunk weights first,
                # then that chunk of x, so projection matmuls for chunk d can
                # start as soon as its inputs land.
                bq_sb = persist.tile([1, E], f32r, tag="bq")
                nc.sync.dma_start(out=bq_sb[:], in_=bq.ap().bitcast(f32r))
                bk_sb = persist.tile([1, E], f32r, tag="bk")
                nc.sync.dma_start(out=bk_sb[:], in_=bk.ap().bitcast(f32r))
                bv_sb = persist.tile([1, 256], f32r, tag="bv")
                nc.sync.dma_start(out=bv_sb[:], in_=bv.ap().bitcast(f32r))
                ones = persist.tile([P, 512], f32r, tag="ones")
                nc.sync.dma_start(out=ones[:], in_=ones_d.ap().bitcast(f32r))

                x_sb, wq_sb, wk_sb, wv_sb = [], [], [], []
                for d in range(KD):
                    t = persist.tile([P, E], f32r, tag=f"wq{d}")
                    nc.sync.dma_start(out=t[:], in_=wqT_d[d].bitcast(f32r))
                    wq_sb.append(t)
                    t = persist.tile([P, E], f32r, tag=f"wk{d}")
                    nc.sync.dma_start(out=t[:], in_=wkT_d[d].bitcast(f32r))
                    wk_sb.append(t)
                    t = persist.tile([P, S], f32r, tag=f"x{d}")
                    nc.sync.dma_start(out=t[:], in_=xT_d[d].bitcast(f32r))
                    x_sb.append(t)
                for d in range(KD):
                    t = persist.tile([P, 256], f32r, tag=f"wv{d}")
                    nc.sync.dma_start(out=t[:], in_=wvT_d[d].bitcast(f32r))
                    wv_sb.append(t)
                wo_a = persist.tile([P, D], f32r, tag="wo_a")
                nc.sync.dma_start(out=wo_a[:], in_=woT.ap()[0:P, :].bitcast(f32r))
                wo_b = persist.tile([64, D], f32r, tag="wo_b")
                nc.sync.dma_start(out=wo_b[:], in_=woT.ap()[P:E, :].bitcast(f32r))

                # preload the exp activation table while the PE chews on
                # projections (first ACTIVATE to a new table set costs ~2.7us)
                et_warm = small.tile([1, 8], f32, tag="warm")
                nc.scalar.activation(et_warm[:], bq_sb[0:1, 0:8], Exp, scale=1.0)

                # PE gap-filler: a bf16 weight tile for dummy LDWEIGHTS
                # (~107ns of always-ready PE work, no PSUM, no consumers).
                # The HAM clock gate re-throttles the PE to half rate after
                # any >=0.5us idle gap and only recovers on ~fully-dense
                # activity windows, so we pad every known PE stall with these.
                warmw = persist.tile([P, P], mybir.dt.bfloat16, tag="warmw")
                nc.vector.tensor_copy(warmw[:], ones[:, 0:P])

                def pe_fill(n):
                    for _ in range(n):
                        nc.tensor.ldweights(warmw[:])

                # keep the PE busy through the DMA lead-in so HAM is warm
                # when the first projection matmuls arrive
                pe_fill(32)

                # ---- persistent activations ----
                qt_a = persist.tile([P, S], f32r, tag="qt_a")   # heads 0,1
                qt_b = persist.tile([64, S], f32r, tag="qt_b")  # head 2
                kt_a = persist.tile([P, S], f32r, tag="kt_a")
                kt_b = persist.tile([64, S], f32r, tag="kt_b")
                v_sb = [persist.tile([P, HG, 65], f32r, tag=f"v{i}", name=f"v{i}") for i in range(SKC)]
                ctx_a = persist.tile([P, S], f32r, tag="ctx_a")
                ctx_b = persist.tile([64, S], f32r, tag="ctx_b")

                # ================= QKV projections =================
                with tc.tile_pool(name="proj_ps", bufs=4, space="PSUM") as proj_ps:
                    # two m-waves of 4 live PSUM tiles each (instead of 8) so
                    # the pool leaves headroom for the vps tag + attention
                    # pools to allocate while late evictions drain
                    for (w_chunks, b_tile, dst_a, dst_b) in (
                        (wq_sb, bq_sb, qt_a, qt_b),
                        (wk_sb, bk_sb, kt_a, kt_b),
                    ):
                        for m in range(2):  # e-tiles: [0:128], [128:192]
                            mw = P if m == 0 else 64
                            ps = [proj_ps.tile([mw, 512], f32, tag="proj", name=f"proj_ps_{m}_{c}")
                                  for c in range(SQC)]
                            for d in range(KD):
                                for c in range(SQC):
                                    nc.tensor.matmul(
                                        ps[c][:],
                                        w_chunks[d][:, m * P : m * P + mw],
                                        x_sb[d][:, c * 512 : (c + 1) * 512],
                                        start=(d == 0), stop=False,
                                    )
                                pe_fill(2)  # absorb DMA jitter between x chunks
                            dst = dst_a if m == 0 else dst_b
                            for c in range(SQC):
                                nc.tensor.matmul(
                                    ps[c][:],
                                    b_tile[0:1, m * P : m * P + mw],
                                    ones[0:1, 0:512],
                                    start=False, stop=True,
                                )
                                nc.vector.tensor_copy(
                                    dst[0:mw, c * 512 : (c + 1) * 512], ps[c][:]
                                )

                    # ---- V projection: dense PE work bridging into attention
                    # (own tag so only 2 PSUM banks stay held late, letting
                    # the attention pools allocate without waiting)
                    for i in range(SKC):
                        vps = proj_ps.tile([P, 256], f32, tag="vps", bufs=2, name=f"vps_{i}")
                        for d in range(KD):
                            nc.tensor.matmul(
                                vps[:],
                                x_sb[d][:, i * P : (i + 1) * P],
                                wv_sb[d][:],
                                start=(d == 0), stop=False,
                            )
                        nc.tensor.matmul(
                            vps[:], ones[0:1, 0:P], bv_sb[0:1, :],
                            start=False, stop=True,
                        )
                        nc.vector.tensor_copy(
                            v_sb[i][:, :, 64:65], ones[:, 0:3][:, :, None]
                        )
                        nc.vector.tensor_copy(
                            v_sb[i][:, :, 0:64],
                            vps[:, 0:E].rearrange("p (h d) -> p h d", h=HG),
                        )

                # ================= attention =================
                with tc.tile_pool(name="sc_ps", bufs=2, space="PSUM") as sc_ps, \
                     tc.tile_pool(name="ctx_ps", bufs=4, space="PSUM") as ctx_ps:
                    for h in range(HG):
                        if h < 2:
                            kt_h = kt_a[h * 64 : (h + 1) * 64, :]
                            qt_h = qt_a[h * 64 : (h + 1) * 64, :]
                            ctx_h = ctx_a[h * 64 : (h + 1) * 64, :]
                        else:
                            kt_h = kt_b[0:64, :]
                            qt_h = qt_b[0:64, :]
                            ctx_h = ctx_b[0:64, :]

                        cps = [ctx_ps.tile([65, 512], f32, tag="ctx", name=f"cps_{h}_{c}") for c in range(SQC)]
                        for i in range(SKC):
                            sps, ets = [], []
                            for half in range(2):
                                sp = sc_ps.tile([P, 1024], f32, tag="sc", name=f"sp_{h}_{i}_{half}")
                                for j in range(2):
                                    nc.tensor.matmul(
                                        sp[:, j * 512 : (j + 1) * 512],
                                        kt_h[:, i * P : (i + 1) * P],
                                        qt_h[:, half * 1024 + j * 512 : half * 1024 + (j + 1) * 512],
                                        start=True, stop=True,
                                    )
                                sps.append(sp)
                            for half in range(2):
                                et = work.tile([P, 1024], f32r, tag="exp", name=f"et_{h}_{i}_{half}")
                                nc.scalar.activation(et[:], sps[half][:], Exp, scale=SCALE)
                                ets.append(et)
                            for half in range(2):
                                for j in range(2):
                                    c = half * 2 + j
                                    nc.tensor.matmul(
                                        cps[c][:],
                                        v_sb[i][:, h, :],
                                        ets[half][:, j * 512 : (j + 1) * 512],
                                        start=(i == 0), stop=(i == SKC - 1),
                                    )
                            pe_fill(4)  # per-iteration PE deficit vs ScalarE exp
                        for c in range(SQC):
                            den = small.tile([1, 512], f32, tag="den")
                            nc.vector.tensor_copy(den[:], cps[c][64:65, :])
                            r = small.tile([1, 512], f32, tag="r")
                            nc.vector.reciprocal_approx_fast(r[:], den[:])
                            rb = small.tile([64, 512], f32, tag="rb")
                            nc.gpsimd.partition_broadcast(rb[:], r[:])
                            nc.vector.tensor_mul(
                                ctx_h[:, c * 512 : (c + 1) * 512],
                                cps[c][0:64, :],
                                rb[:],
                            )
                            if h == HG - 1:
                                # output projection for this sq chunk (all heads done)
                                for e in range(KD):
                                    op = ctx_ps.tile([P, 512], f32, tag="ctx",
                                                     name=f"op_{e}_{c}")
                                    nc.tensor.matmul(
                                        op[:],
                                        wo_a[:, e * P : (e + 1) * P],
                                        ctx_a[:, c * 512 : (c + 1) * 512],
                                        start=True, stop=False,
                                    )
                                    nc.tensor.matmul(
                                        op[:],
                                        wo_b[:, e * P : (e + 1) * P],
                                        ctx_b[:, c * 512 : (c + 1) * 512],
                                        start=False, stop=True,
                                    )
                                    o = work.tile([P, 512], f32, tag="o", bufs=6)
                                    if e % 2 == 0:
                                        nc.vector.tensor_copy(o[:], op[:])
                                    else:
                                        nc.scalar.activation(
                                            o[:], op[:],
                                            mybir.ActivationFunctionType.Copy,
                                        )
                                    nc.sync.dma_start(
                                        out=outT_d[e][:, c * 512 : (c + 1) * 512], in_=o[:]
                                    )
                            if h == HG - 1:
                                pe_fill(6)  # bridge normalize/eviction stalls in the tail

    nc.finalize()
    return nc


def _get_nc(body_reps=1):
    key = ("nc", body_reps)
    if key not in _NC_CACHE:
        _NC_CACHE[key] = _build_bass(body_reps)
    return _NC_CACHE[key]


def _core_inputs(c, x, w_q, b_q, w_k, b_k, w_v, b_v, w_o):
    b, g = divmod(c, 4)
    gs = slice(g * E, (g + 1) * E)
    wv_pad = np.zeros((D, 256), np.float32)
    wv_pad[:, :E] = np.ascontiguousarray(w_v[gs, :].T)
    bv_pad = np.zeros((1, 256), np.float32)
    bv_pad[0, :E] = b_v[gs]
    return {
        "xT": np.ascontiguousarray(x[b].T),
        "wqT": np.ascontiguousarray(w_q[gs, :].T),
        "wkT": np.ascontiguousarray(w_k[gs, :].T),
        "wvT": wv_pad,
        "bq": b_q[gs].reshape(1, E).astype(np.float32),
        "bk": b_k[gs].reshape(1, E).astype(np.float32),
        "bv": bv_pad,
        "woT": np.ascontiguousarray(w_o[:, gs].T),
        "ones": np.ones((P, 512), np.float32),
    }


def kernel(x, w_q, b_q, w_k, b_k, w_v, b_v, w_o, b_o, _trace=False):
    from concourse.bass_utils import run_bass_kernel_spmd

    x = np.asarray(x, np.float32)
    args = [np.asarray(a, np.float32) for a in
            (w_q, b_q, w_k, b_k, w_v, b_v, w_o)]
    b_o = np.asarray(b_o, np.float32)

    nc = _get_nc()
    in_maps = [_core_inputs(c, x, *args) for c in range(8)]
    res = run_bass_kernel_spmd(nc, in_maps, core_ids=list(range(8)), trace=_trace)

    out = np.zeros((B, S, D), np.float32)
    for c in range(8):
        out[c // 4] += res.results[c]["outT"].T
    out += b_o
    if _trace:
        kernel._last_results = res
    return out


# revision 14
# speedup vs baseline: 1.1884x; 1.0263x over previous
"""Multi-head attention (B=2, S=2048, D=768, H=12) on 8 NeuronCores.

Sharding: data-parallel over batch (2) x tensor-parallel over heads (4 groups
of 3 heads) = 8 cores. Each core computes its 3 heads' Q/K/V projections,
attention, and a partial output projection; the host sums the 4 per-batch
partials and adds the output bias.

Per-core kernel layout (all matmuls in float32r: 1 cycle/row at N>=256):
  xT   [768, 2048]  input transposed (d on partitions, 6 chunks of 128)
  QT,KT[192, 2048]  transposed projections (head-major rows, bias via
                    rank-1 ones matmul)
  V    [2048, 3x65] natural-layout V with a ones column appended per head:
                    the ctx matmul lhsT [sk, 65] then yields softmax
                    denominators in PSUM row 64 for free.
  scoresT [sk 128, sq] per (head, sk-chunk) in PSUM -> Exp on ScalarE
                    (scale=1/sqrt(dk) folded into the activation) -> SBUF
  ctxT accumulates over sk in PSUM [65, 512] per sq-chunk; normalized on
                    eviction via reciprocal_approx_fast + partition_broadcast
  outT [768, 2048] partial output projection, host-summed across head groups

Perf notes (vs the first working version):
  - DMAs are emitted weights-first and x interleaved per contraction chunk so
    the first projection matmuls start ~4us in instead of after all loads.
  - The V projection runs as a dense PE block right after the Q/K projection
    matmuls: a PE idle gap >3.4us at the proj->attention boundary re-throttles
    the PE clock to half rate (HAM) for the rest of the kernel.
  - The exp activation table is preloaded during the projection phase.
"""

import sys

sys.path.insert(0, "/opt/trn_rl_repo")

import numpy as np

B, S, D = 2, 2048, 768
H, DK = 12, 64
P = 128
HG = 3              # heads per core
E = HG * DK         # 192: per-core projection width
KD = D // P         # 6 contraction chunks
SQC = S // 512      # 4 sq chunks of 512
SKC = S // P        # 16 sk chunks of 128
SCALE = 1.0 / 8.0   # 1/sqrt(DK)

_NC_CACHE = {}


def _build_bass(body_reps=1):
    import concourse.bacc as bacc
    import concourse.tile as tile
    from concourse import mybir

    f32 = mybir.dt.float32
    f32r = mybir.dt.float32r
    Exp = mybir.ActivationFunctionType.Exp

    nc = bacc.Bacc(trn_type="TRN2", debug=False)

    xT = nc.dram_tensor("xT", [D, S], f32, kind="ExternalInput")
    wqT = nc.dram_tensor("wqT", [D, E], f32, kind="ExternalInput")
    wkT = nc.dram_tensor("wkT", [D, E], f32, kind="ExternalInput")
    wvT = nc.dram_tensor("wvT", [D, 256], f32, kind="ExternalInput")
    bq = nc.dram_tensor("bq", [1, E], f32, kind="ExternalInput")
    bk = nc.dram_tensor("bk", [1, E], f32, kind="ExternalInput")
    bv = nc.dram_tensor("bv", [1, 256], f32, kind="ExternalInput")
    woT = nc.dram_tensor("woT", [E, D], f32, kind="ExternalInput")
    ones_d = nc.dram_tensor("ones", [P, 512], f32, kind="ExternalInput")
    outT = nc.dram_tensor("outT", [D, S], f32, kind="ExternalOutput")

    xT_d = xT.ap().rearrange("(c p) s -> c p s", p=P)
    wqT_d = wqT.ap().rearrange("(c p) e -> c p e", p=P)
    wkT_d = wkT.ap().rearrange("(c p) e -> c p e", p=P)
    wvT_d = wvT.ap().rearrange("(c p) e -> c p e", p=P)
    outT_d = outT.ap().rearrange("(c p) s -> c p s", p=P)

    with tile.TileContext(nc) as tc:
        for _rep in range(body_reps):
            with tc.tile_pool(name="persist", bufs=1) as persist, \
                 tc.tile_pool(name="work", bufs=4) as work, \
                 tc.tile_pool(name="small", bufs=2) as small:

                # ---- load inputs (f32r via dtype-punned DMA: PE truncates).
                # Emission order = DMA queue order: per-chunk weights first,
                # then that chunk of x, so projection matmuls for chunk d can
                # start as soon as its inputs land.
                bq_sb = persist.tile([1, E], f32r, tag="bq")
                nc.sync.dma_start(out=bq_sb[:], in_=bq.ap().bitcast(f32r))
                bk_sb = persist.tile([1, E], f32r, tag="bk")
                nc.sync.dma_start(out=bk_sb[:], in_=bk.ap().bitcast(f32r))
                bv_sb = persist.tile([1, 256], f32r, tag="bv")
                nc.sync.dma_start(out=bv_sb[:], in_=bv.ap().bitcast(f32r))
                ones = persist.tile([P, 512], f32r, tag="ones")
                nc.sync.dma_start(out=ones[:], in_=ones_d.ap().bitcast(f32r))

                x_sb, wq_sb, wk_sb, wv_sb = [], [], [], []
                for d in range(KD):
                    t = persist.tile([P, E], f32r, tag=f"wq{d}")
                    nc.sync.dma_start(out=t[:], in_=wqT_d[d].bitcast(f32r))
                    wq_sb.append(t)
                    t = persist.tile([P, E], f32r, tag=f"wk{d}")
                    nc.sync.dma_start(out=t[:], in_=wkT_d[d].bitcast(f32r))
                    wk_sb.append(t)
                    t = persist.tile([P, S], f32r, tag=f"x{d}")
                    nc.sync.dma_start(out=t[:], in_=xT_d[d].bitcast(f32r))
                    x_sb.append(t)
                for d in range(KD):
                    t = persist.tile([P, 256], f32r, tag=f"wv{d}")
                    nc.sync.dma_start(out=t[:], in_=wvT_d[d].bitcast(f32r))
                    wv_sb.append(t)
                wo_a = persist.tile([P, D], f32r, tag="wo_a")
                nc.sync.dma_start(out=wo_a[:], in_=woT.ap()[0:P, :].bitcast(f32r))
                wo_b = persist.tile([64, D], f32r, tag="wo_b")
                nc.sync.dma_start(out=wo_b[:], in_=woT.ap()[P:E, :].bitcast(f32r))

                # preload the exp activation table while the PE chews on
                # projections (first ACTIVATE to a new table set costs ~2.7us)
                et_warm = small.tile([1, 8], f32, tag="warm")
                nc.scalar.activation(et_warm[:], bq_sb[0:1, 0:8], Exp, scale=1.0)

                # PE gap-filler: a bf16 weight tile for dummy LDWEIGHTS
                # (~107ns of always-ready PE work, no PSUM, no consumers).
                # The HAM clock gate re-throttles the PE to half rate after
                # any >=0.5us idle gap and only recovers on ~fully-dense
                # activity windows, so we pad every known PE stall with these.
                warmw = persist.tile([P, P], mybir.dt.bfloat16, tag="warmw")
                nc.vector.tensor_copy(warmw[:], ones[:, 0:P])

                def pe_fill(n):
                    for _ in range(n):
                        nc.tensor.ldweights(warmw[:])

                # keep the PE busy through the DMA lead-in so HAM is warm
                # when the first projection matmuls arrive
                pe_fill(20)

                # ---- persistent activations ----
                qt_a = persist.tile([P, S], f32r, tag="qt_a")   # heads 0,1
                qt_b = persist.tile([64, S], f32r, tag="qt_b")  # head 2
                kt_a = persist.tile([P, S], f32r, tag="kt_a")
                kt_b = persist.tile([64, S], f32r, tag="kt_b")
                v_sb = [persist.tile([P, HG, 65], f32r, tag=f"v{i}", name=f"v{i}") for i in range(SKC)]
                ctx_a = persist.tile([P, S], f32r, tag="ctx_a")
                ctx_b = persist.tile([64, S], f32r, tag="ctx_b")

                # ================= QKV projections =================
                with tc.tile_pool(name="proj_ps", bufs=4, space="PSUM") as proj_ps:
                    # two m-waves of 4 live PSUM tiles each (instead of 8) so
                    # the pool leaves headroom for the vps tag + attention
                    # pools to allocate while late evictions drain
                    for pi, (w_chunks, b_tile, dst_a, dst_b) in enumerate((
                        (wq_sb, bq_sb, qt_a, qt_b),
                        (wk_sb, bk_sb, kt_a, kt_b),
                    )):
                        for m in range(2):  # e-tiles: [0:128], [128:192]
                            mw = P if m == 0 else 64
                            ps = [proj_ps.tile([mw, 512], f32, tag="proj", name=f"proj_ps_{m}_{c}")
                                  for c in range(SQC)]
                            for d in range(KD):
                                for c in range(SQC):
                                    nc.tensor.matmul(
                                        ps[c][:],
                                        w_chunks[d][:, m * P : m * P + mw],
                                        x_sb[d][:, c * 512 : (c + 1) * 512],
                                        start=(d == 0), stop=False,
                                    )
                                # first pass is x-DMA-paced: PE consumes each
                                # chunk faster than it lands, so pad deeper
                                pe_fill(8 if (pi, m) == (0, 0) else 2)
                            dst = dst_a if m == 0 else dst_b
                            for c in range(SQC):
                                nc.tensor.matmul(
                                    ps[c][:],
                                    b_tile[0:1, m * P : m * P + mw],
                                    ones[0:1, 0:512],
                                    start=False, stop=True,
                                )
                                nc.vector.tensor_copy(
                                    dst[0:mw, c * 512 : (c + 1) * 512], ps[c][:]
                                )

                    # ---- V projection: dense PE work bridging into attention
                    # (own tag so only 2 PSUM banks stay held late, letting
                    # the attention pools allocate without waiting)
                    for i in range(SKC):
                        vps = proj_ps.tile([P, 256], f32, tag="vps", bufs=2, name=f"vps_{i}")
                        for d in range(KD):
                            nc.tensor.matmul(
                                vps[:],
                                x_sb[d][:, i * P : (i + 1) * P],
                                wv_sb[d][:],
                                start=(d == 0), stop=False,
                            )
                        nc.tensor.matmul(
                            vps[:], ones[0:1, 0:P], bv_sb[0:1, :],
                            start=False, stop=True,
                        )
                        nc.vector.tensor_copy(
                            v_sb[i][:, :, 64:65], ones[:, 0:3][:, :, None]
                        )
                        nc.vector.tensor_copy(
                            v_sb[i][:, :, 0:64],
                            vps[:, 0:E].rearrange("p (h d) -> p h d", h=HG),
                        )
                        pe_fill(1)

                # ================= attention =================
                with tc.tile_pool(name="sc_ps", bufs=2, space="PSUM") as sc_ps, \
                     tc.tile_pool(name="ctx_ps", bufs=4, space="PSUM") as ctx_ps:
                    for h in range(HG):
                        if h < 2:
                            kt_h = kt_a[h * 64 : (h + 1) * 64, :]
                            qt_h = qt_a[h * 64 : (h + 1) * 64, :]
                            ctx_h = ctx_a[h * 64 : (h + 1) * 64, :]
                        else:
                            kt_h = kt_b[0:64, :]
                            qt_h = qt_b[0:64, :]
                            ctx_h = ctx_b[0:64, :]

                        cps = [ctx_ps.tile([65, 512], f32, tag="ctx", name=f"cps_{h}_{c}") for c in range(SQC)]
                        pe_fill(6)  # cover the per-head pipeline-fill stall
                        for i in range(SKC):
                            sps, ets = [], []
                            for half in range(2):
                                sp = sc_ps.tile([P, 1024], f32, tag="sc", name=f"sp_{h}_{i}_{half}")
                                for j in range(2):
                                    nc.tensor.matmul(
                                        sp[:, j * 512 : (j + 1) * 512],
                                        kt_h[:, i * P : (i + 1) * P],
                                        qt_h[:, half * 1024 + j * 512 : half * 1024 + (j + 1) * 512],
                                        start=True, stop=True,
                                    )
                                sps.append(sp)
                            # fills sit BETWEEN scores and ctx in the PE
                            # stream: always-ready work at the point where the
                            # PE otherwise waits on the exp results
                            pe_fill(1)
                            for half in range(2):
                                et = work.tile([P, 1024], f32r, tag="exp", name=f"et_{h}_{i}_{half}")
                                nc.scalar.activation(et[:], sps[half][:], Exp, scale=SCALE)
                                ets.append(et)
                            for half in range(2):
                                for j in range(2):
                                    c = half * 2 + j
                                    nc.tensor.matmul(
                                        cps[c][:],
                                        v_sb[i][:, h, :],
                                        ets[half][:, j * 512 : (j + 1) * 512],
                                        start=(i == 0), stop=(i == SKC - 1),
                                    )
                            pe_fill(2)  # per-iteration PE deficit vs ScalarE exp
                        if h == HG - 1:
                            # the h2 normalize chains gate the first output
                            # projection; keep the PE warm across them
                            pe_fill(12)
                        for c in range(SQC):
                            den = small.tile([1, 512], f32, tag="den")
                            nc.vector.tensor_copy(den[:], cps[c][64:65, :])
                            r = small.tile([1, 512], f32, tag="r")
                            nc.vector.reciprocal_approx_fast(r[:], den[:])
                            rb = small.tile([64, 512], f32, tag="rb")
                            nc.gpsimd.partition_broadcast(rb[:], r[:])
                            nc.vector.tensor_mul(
                                ctx_h[:, c * 512 : (c + 1) * 512],
                                cps[c][0:64, :],
                                rb[:],
                            )
                            if h == HG - 1:
                                # output projection for this sq chunk (all heads done)
                                for e in range(KD):
                                    op = ctx_ps.tile([P, 512], f32, tag="ctx",
                                                     name=f"op_{e}_{c}")
                                    nc.tensor.matmul(
                                        op[:],
                                        wo_a[:, e * P : (e + 1) * P],
                                        ctx_a[:, c * 512 : (c + 1) * 512],
                                        start=True, stop=False,
                                    )
                                    nc.tensor.matmul(
                                        op[:],
                                        wo_b[:, e * P : (e + 1) * P],
                                        ctx_b[:, c * 512 : (c + 1) * 512],
                                        start=False, stop=True,
                                    )
                                    o = work.tile([P, 512], f32, tag="o", bufs=6)
                                    if e % 2 == 0:
                                        nc.vector.tensor_copy(o[:], op[:])
                                    else:
                                        nc.scalar.activation(
                                            o[:], op[:],
                                            mybir.ActivationFunctionType.Copy,
                                        )
                                    nc.sync.dma_start(
                                        out=outT_d[e][:, c * 512 : (c + 1) * 512], in_=o[:]
                                    )
                            if h == HG - 1:
                                pe_fill(6)  # bridge normalize/eviction stalls in the tail

    nc.finalize()
    return nc


def _get_nc(body_reps=1):
    key = ("nc", body_reps)
    if key not in _NC_CACHE:
        _NC_CACHE[key] = _build_bass(body_reps)
    return _NC_CACHE[key]


def _core_inputs(c, x, w_q, b_q, w_k, b_k, w_v, b_v, w_o):
    b, g = divmod(c, 4)
    gs = slice(g * E, (g + 1) * E)
    wv_pad = np.zeros((D, 256), np.float32)
    wv_pad[:, :E] = np.ascontiguousarray(w_v[gs, :].T)
    bv_pad = np.zeros((1, 256), np.float32)
    bv_pad[0, :E] = b_v[gs]
    return {
        "xT": np.ascontiguousarray(x[b].T),
        "wqT": np.ascontiguousarray(w_q[gs, :].T),
        "wkT": np.ascontiguousarray(w_k[gs, :].T),
        "wvT": wv_pad,
        "bq": b_q[gs].reshape(1, E).astype(np.float32),
        "bk": b_k[gs].reshape(1, E).astype(np.float32),
        "bv": bv_pad,
        "woT": np.ascontiguousarray(w_o[:, gs].T),
        "ones": np.ones((P, 512), np.float32),
    }


def kernel(x, w_q, b_q, w_k, b_k, w_v, b_v, w_o, b_o, _trace=False):
    from concourse.bass_utils import run_bass_kernel_spmd

    x = np.asarray(x, np.float32)
    args = [np.asarray(a, np.float32) for a in
            (w_q, b_q, w_k, b_k, w_v, b_v, w_o)]
    b_o = np.asarray(b_o, np.float32)

    nc = _get_nc()
    in_maps = [_core_inputs(c, x, *args) for c in range(8)]
    res = run_bass_kernel_spmd(nc, in_maps, core_ids=list(range(8)), trace=_trace)

    out = np.zeros((B, S, D), np.float32)
    for c in range(8):
        out[c // 4] += res.results[c]["outT"].T
    out += b_o
    if _trace:
        kernel._last_results = res
    return out


# revision 15
# speedup vs baseline: 1.3064x; 1.0993x over previous
"""Multi-head attention (B=2, S=2048, D=768, H=12) on 8 NeuronCores.

Sharding: data-parallel over batch (2) x tensor-parallel over heads (4 groups
of 3 heads) = 8 cores. Each core computes its 3 heads' Q/K/V projections,
attention, and a partial output projection; the host sums the 4 per-batch
partials and adds the output bias.

Per-core kernel layout:
  xT   [768, 2048]  input transposed, bf16 (d on partitions, 6 chunks of 128)
  QT,KT[192, 2048]  transposed projections, f32r (head-major rows, bias via
                    rank-1 ones matmul; bf16 inputs, fp32 PSUM accumulation)
  V    [2048, 3x65] natural-layout f32r V with a ones column appended per
                    head: the ctx matmul lhsT [sk, 65] then yields softmax
                    denominators in PSUM row 64 for free.
  Attention: per head, per sq-chunk-pair (2x512), loop sk chunks:
    scoresT [sk 128, sq 1024] in PSUM -> Exp on ScalarE (scale=1/sqrt(dk)
    folded) -> f32r SBUF -> two ctx matmuls accumulate [65, 512] per chunk.
    The chunk-pair split keeps only 2 ctx banks live so the score pool gets
    3 bufs (6 banks) = 3 iterations of PE run-ahead over the exp pipeline.
  outT [768, 2048] partial output projection per sq chunk at the last head,
                    host-summed across head groups.

HAM note: the PE clock gate throttles to half rate after any >=0.5us PE idle
gap and only recovers on near-fully-dense activity windows. All known stall
points are padded with dummy bf16 LDWEIGHTS (~100ns each, always ready, no
consumers): the DMA lead-in, x-DMA-paced projection chunks, the per-iteration
exp-wait slot, head boundaries, and the h2 normalize tail.
"""

import sys

sys.path.insert(0, "/opt/trn_rl_repo")

import numpy as np

B, S, D = 2, 2048, 768
H, DK = 12, 64
P = 128
HG = 3              # heads per core
E = HG * DK         # 192: per-core projection width
KD = D // P         # 6 contraction chunks
SQC = S // 512      # 4 sq chunks of 512
SKC = S // P        # 16 sk chunks of 128
SCALE = 1.0 / 8.0   # 1/sqrt(DK)

_NC_CACHE = {}


def _build_bass(body_reps=1):
    import concourse.bacc as bacc
    import concourse.tile as tile
    from concourse import mybir

    f32 = mybir.dt.float32
    f32r = mybir.dt.float32r
    bf16 = mybir.dt.bfloat16
    Exp = mybir.ActivationFunctionType.Exp

    nc = bacc.Bacc(trn_type="TRN2", debug=False)

    xT = nc.dram_tensor("xT", [D, S], bf16, kind="ExternalInput")
    wqT = nc.dram_tensor("wqT", [D, E], bf16, kind="ExternalInput")
    wkT = nc.dram_tensor("wkT", [D, E], bf16, kind="ExternalInput")
    wvT = nc.dram_tensor("wvT", [D, 256], bf16, kind="ExternalInput")
    bq = nc.dram_tensor("bq", [1, E], bf16, kind="ExternalInput")
    bk = nc.dram_tensor("bk", [1, E], bf16, kind="ExternalInput")
    bv = nc.dram_tensor("bv", [1, 256], bf16, kind="ExternalInput")
    woT = nc.dram_tensor("woT", [E, D], f32, kind="ExternalInput")
    ones_d = nc.dram_tensor("ones", [P, 512], f32, kind="ExternalInput")
    outT = nc.dram_tensor("outT", [D, S], f32, kind="ExternalOutput")

    xT_d = xT.ap().rearrange("(c p) s -> c p s", p=P)
    wqT_d = wqT.ap().rearrange("(c p) e -> c p e", p=P)
    wkT_d = wkT.ap().rearrange("(c p) e -> c p e", p=P)
    wvT_d = wvT.ap().rearrange("(c p) e -> c p e", p=P)
    outT_d = outT.ap().rearrange("(c p) s -> c p s", p=P)

    with tile.TileContext(nc) as tc:
        for _rep in range(body_reps):
            with tc.tile_pool(name="persist", bufs=1) as persist, \
                 tc.tile_pool(name="work", bufs=4) as work, \
                 tc.tile_pool(name="small", bufs=2) as small:

                # ---- load inputs. Emission order = DMA queue order:
                # per-chunk weights just before that chunk of x, so
                # projection matmuls for chunk d start as soon as it lands.
                bq_sb = persist.tile([1, E], bf16, tag="bq")
                nc.sync.dma_start(out=bq_sb[:], in_=bq.ap())
                bk_sb = persist.tile([1, E], bf16, tag="bk")
                nc.sync.dma_start(out=bk_sb[:], in_=bk.ap())
                bv_sb = persist.tile([1, 256], bf16, tag="bv")
                nc.sync.dma_start(out=bv_sb[:], in_=bv.ap())
                ones = persist.tile([P, 512], f32r, tag="ones")
                nc.sync.dma_start(out=ones[:], in_=ones_d.ap().bitcast(f32r))

                x_sb, wq_sb, wk_sb, wv_sb = [], [], [], []
                for d in range(KD):
                    t = persist.tile([P, E], bf16, tag=f"wq{d}")
                    nc.sync.dma_start(out=t[:], in_=wqT_d[d])
                    wq_sb.append(t)
                    t = persist.tile([P, E], bf16, tag=f"wk{d}")
                    nc.sync.dma_start(out=t[:], in_=wkT_d[d])
                    wk_sb.append(t)
                    t = persist.tile([P, S], bf16, tag=f"x{d}")
                    nc.sync.dma_start(out=t[:], in_=xT_d[d])
                    x_sb.append(t)
                for d in range(KD):
                    t = persist.tile([P, 256], bf16, tag=f"wv{d}")
                    nc.sync.dma_start(out=t[:], in_=wvT_d[d])
                    wv_sb.append(t)
                wo_a = persist.tile([P, D], f32r, tag="wo_a")
                nc.sync.dma_start(out=wo_a[:], in_=woT.ap()[0:P, :].bitcast(f32r))
                wo_b = persist.tile([64, D], f32r, tag="wo_b")
                nc.sync.dma_start(out=wo_b[:], in_=woT.ap()[P:E, :].bitcast(f32r))

                # bf16 ones row for the rank-1 bias matmuls (dtypes of the
                # two matmul operands must match)
                ones_bf = persist.tile([1, 512], bf16, tag="ones_bf")
                nc.vector.tensor_copy(ones_bf[:], ones[0:1, :])

                # preload the exp activation table while the PE chews on
                # projections (first ACTIVATE to a new table set costs ~2.7us)
                et_warm = small.tile([1, 8], f32, tag="warm")
                nc.scalar.activation(et_warm[:], ones[0:1, 0:8], Exp, scale=1.0)

                # PE gap-filler: dummy bf16 LDWEIGHTS (~100ns each)
                warmw = persist.tile([P, P], bf16, tag="warmw")
                nc.vector.tensor_copy(warmw[:], ones[:, 0:P])

                def pe_fill(n):
                    for _ in range(n):
                        nc.tensor.ldweights(warmw[:])

                # keep the PE busy through the DMA lead-in so HAM is warm
                # when the first projection matmuls arrive
                pe_fill(20)

                # ---- persistent activations ----
                qt_a = persist.tile([P, S], f32r, tag="qt_a")   # heads 0,1
                qt_b = persist.tile([64, S], f32r, tag="qt_b")  # head 2
                kt_a = persist.tile([P, S], f32r, tag="kt_a")
                kt_b = persist.tile([64, S], f32r, tag="kt_b")
                v_sb = [persist.tile([P, HG, 65], f32r, tag=f"v{i}", name=f"v{i}") for i in range(SKC)]
                ctx_a = persist.tile([P, S], f32r, tag="ctx_a")
                ctx_b = persist.tile([64, S], f32r, tag="ctx_b")

                # ================= QKV projections =================
                with tc.tile_pool(name="proj_ps", bufs=4, space="PSUM") as proj_ps:
                    # two m-waves of 4 live PSUM tiles each so the pool stays
                    # at 6 banks total (with vps) and the attention pools can
                    # allocate while late evictions drain
                    for pi, (w_chunks, b_tile, dst_a, dst_b) in enumerate((
                        (wq_sb, bq_sb, qt_a, qt_b),
                        (wk_sb, bk_sb, kt_a, kt_b),
                    )):
                        for m in range(2):  # e-tiles: [0:128], [128:192]
                            mw = P if m == 0 else 64
                            ps = [proj_ps.tile([mw, 512], f32, tag="proj", name=f"proj_ps_{m}_{c}")
                                  for c in range(SQC)]
                            for d in range(KD):
                                for c in range(SQC):
                                    nc.tensor.matmul(
                                        ps[c][:],
                                        w_chunks[d][:, m * P : m * P + mw],
                                        x_sb[d][:, c * 512 : (c + 1) * 512],
                                        start=(d == 0), stop=False,
                                    )
                                # first pass is x-DMA-paced; pad the wait
                                pe_fill(6 if (pi, m) == (0, 0) else 2)
                            dst = dst_a if m == 0 else dst_b
                            for c in range(SQC):
                                nc.tensor.matmul(
                                    ps[c][:],
                                    b_tile[0:1, m * P : m * P + mw],
                                    ones_bf[0:1, 0:512],
                                    start=False, stop=True,
                                )
                                nc.vector.tensor_copy(
                                    dst[0:mw, c * 512 : (c + 1) * 512], ps[c][:]
                                )

                    # ---- V projection: dense PE work bridging into attention
                    for i in range(SKC):
                        vps = proj_ps.tile([P, 256], f32, tag="vps", bufs=2, name=f"vps_{i}")
                        for d in range(KD):
                            nc.tensor.matmul(
                                vps[:],
                                x_sb[d][:, i * P : (i + 1) * P],
                                wv_sb[d][:],
                                start=(d == 0), stop=False,
                            )
                        nc.tensor.matmul(
                            vps[:], ones_bf[0:1, 0:P], bv_sb[0:1, :],
                            start=False, stop=True,
                        )
                        nc.vector.tensor_copy(
                            v_sb[i][:, :, 64:65], ones[:, 0:3][:, :, None]
                        )
                        nc.vector.tensor_copy(
                            v_sb[i][:, :, 0:64],
                            vps[:, 0:E].rearrange("p (h d) -> p h d", h=HG),
                        )
                        pe_fill(1)

                # ================= attention =================
                # per (head, sq-chunk-pair): 16 sk iterations of
                #   scores [128,1024] -> exp -> 2x ctx accumulate [65,512]
                with tc.tile_pool(name="sc_ps", bufs=3, space="PSUM") as sc_ps, \
                     tc.tile_pool(name="ctx_ps", bufs=2, space="PSUM") as ctx_ps:
                    for h in range(HG):
                        if h < 2:
                            kt_h = kt_a[h * 64 : (h + 1) * 64, :]
                            qt_h = qt_a[h * 64 : (h + 1) * 64, :]
                            ctx_h = ctx_a[h * 64 : (h + 1) * 64, :]
                        else:
                            kt_h = kt_b[0:64, :]
                            qt_h = qt_b[0:64, :]
                            ctx_h = ctx_b[0:64, :]

                        for g in range(2):  # sq-chunk pair: chunks 2g, 2g+1
                            cps = [ctx_ps.tile([65, 512], f32, tag="ctx",
                                               name=f"cps_{h}_{g}_{j}")
                                   for j in range(2)]
                            pe_fill(4)  # pipeline-fill stall at (head, pair) start
                            for i in range(SKC):
                                sp = sc_ps.tile([P, 1024], f32, tag="sc",
                                                name=f"sp_{h}_{g}_{i}")
                                for j in range(2):
                                    nc.tensor.matmul(
                                        sp[:, j * 512 : (j + 1) * 512],
                                        kt_h[:, i * P : (i + 1) * P],
                                        qt_h[:, g * 1024 + j * 512 : g * 1024 + (j + 1) * 512],
                                        start=True, stop=True,
                                    )
                                # fills sit between scores and ctx in the PE
                                # stream: ready work right where the PE waits
                                # on the exp result
                                pe_fill(1)
                                et = work.tile([P, 1024], f32r, tag="exp",
                                               name=f"et_{h}_{g}_{i}")
                                nc.scalar.activation(et[:], sp[:], Exp, scale=SCALE)
                                for j in range(2):
                                    nc.tensor.matmul(
                                        cps[j][:],
                                        v_sb[i][:, h, :],
                                        et[:, j * 512 : (j + 1) * 512],
                                        start=(i == 0), stop=(i == SKC - 1),
                                    )
                                pe_fill(1)
                            if h == HG - 1:
                                pe_fill(8)  # normalize chains gate the out-proj
                            for j in range(2):
                                c = 2 * g + j
                                den = small.tile([1, 512], f32, tag="den")
                                nc.vector.tensor_copy(den[:], cps[j][64:65, :])
                                r = small.tile([1, 512], f32, tag="r")
                                nc.vector.reciprocal_approx_fast(r[:], den[:])
                                rb = small.tile([64, 512], f32, tag="rb")
                                nc.gpsimd.partition_broadcast(rb[:], r[:])
                                nc.vector.tensor_mul(
                                    ctx_h[:, c * 512 : (c + 1) * 512],
                                    cps[j][0:64, :],
                                    rb[:],
                                )
                                if h == HG - 1:
                                    # output projection for this sq chunk
                                    for e in range(KD):
                                        op = ctx_ps.tile([P, 512], f32, tag="ctx",
                                                         name=f"op_{e}_{c}")
                                        nc.tensor.matmul(
                                            op[:],
                                            wo_a[:, e * P : (e + 1) * P],
                                            ctx_a[:, c * 512 : (c + 1) * 512],
                                            start=True, stop=False,
                                        )
                                        nc.tensor.matmul(
                                            op[:],
                                            wo_b[:, e * P : (e + 1) * P],
                                            ctx_b[:, c * 512 : (c + 1) * 512],
                                            start=False, stop=True,
                                        )
                                        o = work.tile([P, 512], f32, tag="o", bufs=6)
                                        if e % 2 == 0:
                                            nc.vector.tensor_copy(o[:], op[:])
                                        else:
                                            nc.scalar.activation(
                                                o[:], op[:],
                                                mybir.ActivationFunctionType.Copy,
                                            )
                                        nc.sync.dma_start(
                                            out=outT_d[e][:, c * 512 : (c + 1) * 512], in_=o[:]
                                        )
                                    pe_fill(4)

    nc.finalize()
    return nc


def _get_nc(body_reps=1):
    key = ("nc", body_reps)
    if key not in _NC_CACHE:
        _NC_CACHE[key] = _build_bass(body_reps)
    return _NC_CACHE[key]


def _core_inputs(c, x, w_q, b_q, w_k, b_k, w_v, b_v, w_o):
    import ml_dtypes
    bf = ml_dtypes.bfloat16
    b, g = divmod(c, 4)
    gs = slice(g * E, (g + 1) * E)
    wv_pad = np.zeros((D, 256), bf)
    wv_pad[:, :E] = np.ascontiguousarray(w_v[gs, :].T).astype(bf)
    bv_pad = np.zeros((1, 256), bf)
    bv_pad[0, :E] = b_v[gs].astype(bf)
    return {
        "xT": np.ascontiguousarray(x[b].T).astype(bf),
        "wqT": np.ascontiguousarray(w_q[gs, :].T).astype(bf),
        "wkT": np.ascontiguousarray(w_k[gs, :].T).astype(bf),
        "wvT": wv_pad,
        "bq": b_q[gs].reshape(1, E).astype(bf),
        "bk": b_k[gs].reshape(1, E).astype(bf),
        "bv": bv_pad,
        "woT": np.ascontiguousarray(w_o[:, gs].T),
        "ones": np.ones((P, 512), np.float32),
    }


def kernel(x, w_q, b_q, w_k, b_k, w_v, b_v, w_o, b_o, _trace=False):
    from concourse.bass_utils import run_bass_kernel_spmd

    x = np.asarray(x, np.float32)
    args = [np.asarray(a, np.float32) for a in
            (w_q, b_q, w_k, b_k, w_v, b_v, w_o)]
    b_o = np.asarray(b_o, np.float32)

    nc = _get_nc()
    in_maps = [_core_inputs(c, x, *args) for c in range(8)]
    res = run_bass_kernel_spmd(nc, in_maps, core_ids=list(range(8)), trace=_trace)

    out = np.zeros((B, S, D), np.float32)
    for c in range(8):
        out[c // 4] += res.results[c]["outT"].T
    out += b_o
    if _trace:
        kernel._last_results = res
    return out


# revision 17
# speedup vs baseline: 1.4669x; 1.1228x over previous
"""Multi-head attention (B=2, S=2048, D=768, H=12) on 8 NeuronCores.

Sharding: data-parallel over batch (2) x tensor-parallel over heads (4 groups
of 3 heads) = 8 cores. Each core computes its 3 heads' Q/K/V projections,
attention, and a partial output projection; the host sums the 4 per-batch
partials and adds the output bias.

Per-core kernel layout:
  xT   [768, 2048]  input transposed, bf16 (d on partitions, 6 chunks of 128)
  QT,KT[192, 2048]  transposed projections, f32r (head-major rows, bias via
                    rank-1 ones matmul; bf16 inputs, fp32 PSUM accumulation)
  V    [2048, 3x65] natural-layout f32r V with a ones column appended per
                    head: the ctx matmul lhsT [sk, 65] then yields softmax
                    denominators in PSUM row 64 for free.
  Attention: per head, per sq-chunk-pair (2x512), loop sk chunks:
    scoresT [sk 128, sq 1024] in PSUM -> Exp on ScalarE (scale=1/sqrt(dk)
    folded) -> f32r SBUF -> two ctx matmuls accumulate [65, 512] per chunk.
    The chunk-pair split keeps only 2 ctx banks live so the score pool gets
    3 bufs (6 banks) = 3 iterations of PE run-ahead over the exp pipeline.
  outT [768, 2048] partial output projection per sq chunk at the last head,
                    host-summed across head groups.

HAM note: the PE clock gate throttles to half rate after any >=0.5us PE idle
gap and only recovers on near-fully-dense activity windows. All known stall
points are padded with dummy bf16 LDWEIGHTS (~100ns each, always ready, no
consumers): the DMA lead-in, x-DMA-paced projection chunks, the per-iteration
exp-wait slot, head boundaries, and the h2 normalize tail.
"""

import sys

sys.path.insert(0, "/opt/trn_rl_repo")

import numpy as np

B, S, D = 2, 2048, 768
H, DK = 12, 64
P = 128
HG = 3              # heads per core
E = HG * DK         # 192: per-core projection width
KD = D // P         # 6 contraction chunks
SQC = S // 512      # 4 sq chunks of 512
SKC = S // P        # 16 sk chunks of 128
SCALE = 1.0 / 8.0   # 1/sqrt(DK)

_NC_CACHE = {}


def _build_bass(body_reps=1):
    import concourse.bacc as bacc
    import concourse.tile as tile
    from concourse import mybir

    f32 = mybir.dt.float32
    f32r = mybir.dt.float32r
    bf16 = mybir.dt.bfloat16
    Exp = mybir.ActivationFunctionType.Exp

    nc = bacc.Bacc(trn_type="TRN2", debug=False)

    xT = nc.dram_tensor("xT", [D, S], bf16, kind="ExternalInput")
    wqT = nc.dram_tensor("wqT", [D, E], bf16, kind="ExternalInput")
    wkT = nc.dram_tensor("wkT", [D, E], bf16, kind="ExternalInput")
    wvT = nc.dram_tensor("wvT", [D, 256], bf16, kind="ExternalInput")
    bq = nc.dram_tensor("bq", [1, E], bf16, kind="ExternalInput")
    bk = nc.dram_tensor("bk", [1, E], bf16, kind="ExternalInput")
    bv = nc.dram_tensor("bv", [1, 256], bf16, kind="ExternalInput")
    woT = nc.dram_tensor("woT", [E, D], f32, kind="ExternalInput")
    ones_d = nc.dram_tensor("ones", [P, 512], f32, kind="ExternalInput")
    outT = nc.dram_tensor("outT", [D, S], f32, kind="ExternalOutput")

    xT_d = xT.ap().rearrange("(c p) s -> c p s", p=P)
    wqT_d = wqT.ap().rearrange("(c p) e -> c p e", p=P)
    wkT_d = wkT.ap().rearrange("(c p) e -> c p e", p=P)
    wvT_d = wvT.ap().rearrange("(c p) e -> c p e", p=P)
    outT_d = outT.ap().rearrange("(c p) s -> c p s", p=P)

    with tile.TileContext(nc) as tc:
        for _rep in range(body_reps):
            with tc.tile_pool(name="persist", bufs=1) as persist, \
                 tc.tile_pool(name="work", bufs=4) as work, \
                 tc.tile_pool(name="small", bufs=2) as small:

                # ---- load inputs. Emission order = DMA queue order:
                # per-chunk weights just before that chunk of x, so
                # projection matmuls for chunk d start as soon as it lands.
                bq_sb = persist.tile([1, E], bf16, tag="bq")
                nc.sync.dma_start(out=bq_sb[:], in_=bq.ap())
                bk_sb = persist.tile([1, E], bf16, tag="bk")
                nc.sync.dma_start(out=bk_sb[:], in_=bk.ap())
                bv_sb = persist.tile([1, 256], bf16, tag="bv")
                nc.sync.dma_start(out=bv_sb[:], in_=bv.ap())
                ones = persist.tile([P, 512], f32r, tag="ones")
                nc.sync.dma_start(out=ones[:], in_=ones_d.ap().bitcast(f32r))

                x_sb, wq_sb, wk_sb, wv_sb = [], [], [], []
                for d in range(KD):
                    t = persist.tile([P, E], bf16, tag=f"wq{d}")
                    nc.sync.dma_start(out=t[:], in_=wqT_d[d])
                    wq_sb.append(t)
                    t = persist.tile([P, E], bf16, tag=f"wk{d}")
                    nc.sync.dma_start(out=t[:], in_=wkT_d[d])
                    wk_sb.append(t)
                    t = persist.tile([P, S], bf16, tag=f"x{d}")
                    nc.sync.dma_start(out=t[:], in_=xT_d[d])
                    x_sb.append(t)
                for d in range(KD):
                    t = persist.tile([P, 256], bf16, tag=f"wv{d}")
                    nc.sync.dma_start(out=t[:], in_=wvT_d[d])
                    wv_sb.append(t)
                wo_a = persist.tile([P, D], f32r, tag="wo_a")
                nc.sync.dma_start(out=wo_a[:], in_=woT.ap()[0:P, :].bitcast(f32r))
                wo_b = persist.tile([64, D], f32r, tag="wo_b")
                nc.sync.dma_start(out=wo_b[:], in_=woT.ap()[P:E, :].bitcast(f32r))

                # bf16 ones row for the rank-1 bias matmuls (dtypes of the
                # two matmul operands must match)
                ones_bf = persist.tile([1, 512], bf16, tag="ones_bf")
                nc.vector.tensor_copy(ones_bf[:], ones[0:1, :])

                # preload the exp activation table while the PE chews on
                # projections (first ACTIVATE to a new table set costs ~2.7us)
                et_warm = small.tile([1, 8], f32, tag="warm")
                nc.scalar.activation(et_warm[:], ones[0:1, 0:8], Exp, scale=1.0)

                # PE gap-filler: dummy bf16 LDWEIGHTS (~100ns each)
                warmw = persist.tile([P, P], bf16, tag="warmw")
                nc.vector.tensor_copy(warmw[:], ones[:, 0:P])

                def pe_fill(n):
                    for _ in range(n):
                        nc.tensor.ldweights(warmw[:])

                # keep the PE busy through the DMA lead-in so HAM is warm
                # when the first projection matmuls arrive
                pe_fill(20)

                # ---- persistent activations ----
                qt_a = persist.tile([P, S], f32r, tag="qt_a")   # heads 0,1
                qt_b = persist.tile([64, S], f32r, tag="qt_b")  # head 2
                kt_a = persist.tile([P, S], f32r, tag="kt_a")
                kt_b = persist.tile([64, S], f32r, tag="kt_b")
                v_sb = [persist.tile([P, HG, 65], f32r, tag=f"v{i}", name=f"v{i}") for i in range(SKC)]
                ctx_a = persist.tile([P, S], f32r, tag="ctx_a")
                ctx_b = persist.tile([64, S], f32r, tag="ctx_b")

                # ================= QKV projections =================
                with tc.tile_pool(name="proj_ps", bufs=4, space="PSUM") as proj_ps:
                    # two m-waves of 4 live PSUM tiles each so the pool stays
                    # at 6 banks total (with vps) and the attention pools can
                    # allocate while late evictions drain
                    for pi, (w_chunks, b_tile, dst_a, dst_b) in enumerate((
                        (wq_sb, bq_sb, qt_a, qt_b),
                        (wk_sb, bk_sb, kt_a, kt_b),
                    )):
                        for m in range(2):  # e-tiles: [0:128], [128:192]
                            mw = P if m == 0 else 64
                            ps = [proj_ps.tile([mw, 512], f32, tag="proj", name=f"proj_ps_{m}_{c}")
                                  for c in range(SQC)]
                            for d in range(KD):
                                for c in range(SQC):
                                    nc.tensor.matmul(
                                        ps[c][:],
                                        w_chunks[d][:, m * P : m * P + mw],
                                        x_sb[d][:, c * 512 : (c + 1) * 512],
                                        start=(d == 0), stop=False,
                                    )
                                # first pass is x-DMA-paced; pad the wait
                                pe_fill(6 if (pi, m) == (0, 0) else 2)
                            dst = dst_a if m == 0 else dst_b
                            for c in range(SQC):
                                nc.tensor.matmul(
                                    ps[c][:],
                                    b_tile[0:1, m * P : m * P + mw],
                                    ones_bf[0:1, 0:512],
                                    start=False, stop=True,
                                )
                                nc.vector.tensor_copy(
                                    dst[0:mw, c * 512 : (c + 1) * 512], ps[c][:]
                                )

                    # ---- V projection: dense PE work bridging into attention
                    for i in range(SKC):
                        vps = proj_ps.tile([P, 256], f32, tag="vps", bufs=2, name=f"vps_{i}")
                        for d in range(KD):
                            nc.tensor.matmul(
                                vps[:],
                                x_sb[d][:, i * P : (i + 1) * P],
                                wv_sb[d][:],
                                start=(d == 0), stop=False,
                            )
                        nc.tensor.matmul(
                            vps[:], ones_bf[0:1, 0:P], bv_sb[0:1, :],
                            start=False, stop=True,
                        )
                        nc.vector.tensor_copy(
                            v_sb[i][:, :, 64:65], ones[:, 0:3][:, :, None]
                        )
                        nc.vector.tensor_copy(
                            v_sb[i][:, :, 0:64],
                            vps[:, 0:E].rearrange("p (h d) -> p h d", h=HG),
                        )
                        pe_fill(1)

                # ================= attention =================
                # per (head, sq-chunk-pair): 16 sk iterations of
                #   scores [128,1024] -> exp -> 2x ctx accumulate [65,512]
                with tc.tile_pool(name="sc_ps", bufs=3, space="PSUM") as sc_ps, \
                     tc.tile_pool(name="ctx_ps", bufs=2, space="PSUM") as ctx_ps:
                    # Heads 0,1 run PAIRED: their kt/qt live at partitions
                    # 0-63 / 64-127 of the same tiles, so the two score
                    # matmuls row-tile to (0,0) and (64,0) and run
                    # concurrently in the PE array (~2x on scores).
                    # Loop sq chunks; per (c, i): scores [128, h0c|h1c],
                    # one exp, two ctx accumulations.
                    for c in range(SQC):
                        cps = [ctx_ps.tile([65, 512], f32, tag="ctx",
                                           name=f"cps01_{c}_{j}")
                               for j in range(2)]
                        pe_fill(4)
                        for i in range(SKC):
                            sp = sc_ps.tile([P, 1024], f32, tag="sc",
                                            name=f"sp01_{c}_{i}")
                            for j in range(2):  # j = head 0, 1
                                nc.tensor.matmul(
                                    sp[:, j * 512 : (j + 1) * 512],
                                    kt_a[j * 64 : (j + 1) * 64, i * P : (i + 1) * P],
                                    qt_a[j * 64 : (j + 1) * 64, c * 512 : (c + 1) * 512],
                                    start=True, stop=True,
                                )
                            # fills sit between scores and ctx in the PE
                            # stream: ready work right where the PE waits on
                            # the exp result (deeper on the last iteration,
                            # where no ctx work remains to overlap the exp)
                            pe_fill(3 if i < SKC - 1 else 8)
                            et = work.tile([P, 1024], f32r, tag="exp",
                                           name=f"et01_{c}_{i}")
                            nc.scalar.activation(et[:], sp[:], Exp, scale=SCALE)
                            for j in range(2):
                                nc.tensor.matmul(
                                    cps[j][:],
                                    v_sb[i][:, j, :],
                                    et[:, j * 512 : (j + 1) * 512],
                                    start=(i == 0), stop=(i == SKC - 1),
                                )
                            pe_fill(1)
                        for j in range(2):
                            den = small.tile([1, 512], f32, tag="den")
                            nc.vector.tensor_copy(den[:], cps[j][64:65, :])
                            r = small.tile([1, 512], f32, tag="r")
                            nc.vector.reciprocal_approx_fast(r[:], den[:])
                            rb = small.tile([64, 512], f32, tag="rb")
                            nc.gpsimd.partition_broadcast(rb[:], r[:])
                            nc.vector.tensor_mul(
                                ctx_a[j * 64 : (j + 1) * 64, c * 512 : (c + 1) * 512],
                                cps[j][0:64, :],
                                rb[:],
                            )
                    # Head 2 runs solo over sq-chunk pairs.
                    kt_h = kt_b[0:64, :]
                    qt_h = qt_b[0:64, :]
                    ctx_h = ctx_b[0:64, :]
                    for g in range(2):  # sq-chunk pair: chunks 2g, 2g+1
                        cps = [ctx_ps.tile([65, 512], f32, tag="ctx",
                                           name=f"cps2_{g}_{j}")
                               for j in range(2)]
                        pe_fill(4)
                        for i in range(SKC):
                            sp = sc_ps.tile([P, 1024], f32, tag="sc",
                                            name=f"sp2_{g}_{i}")
                            for j in range(2):
                                nc.tensor.matmul(
                                    sp[:, j * 512 : (j + 1) * 512],
                                    kt_h[:, i * P : (i + 1) * P],
                                    qt_h[:, g * 1024 + j * 512 : g * 1024 + (j + 1) * 512],
                                    start=True, stop=True,
                                )
                            pe_fill(1 if i < SKC - 1 else 8)
                            et = work.tile([P, 1024], f32r, tag="exp",
                                           name=f"et2_{g}_{i}")
                            nc.scalar.activation(et[:], sp[:], Exp, scale=SCALE)
                            for j in range(2):
                                nc.tensor.matmul(
                                    cps[j][:],
                                    v_sb[i][:, 2, :],
                                    et[:, j * 512 : (j + 1) * 512],
                                    start=(i == 0), stop=(i == SKC - 1),
                                )
                            pe_fill(1)
                        pe_fill(8)  # normalize chains gate the out-proj
                        for j in range(2):
                            c = 2 * g + j
                            den = small.tile([1, 512], f32, tag="den")
                            nc.vector.tensor_copy(den[:], cps[j][64:65, :])
                            r = small.tile([1, 512], f32, tag="r")
                            nc.vector.reciprocal_approx_fast(r[:], den[:])
                            rb = small.tile([64, 512], f32, tag="rb")
                            nc.gpsimd.partition_broadcast(rb[:], r[:])
                            nc.vector.tensor_mul(
                                ctx_h[:, c * 512 : (c + 1) * 512],
                                cps[j][0:64, :],
                                rb[:],
                            )
                            # output projection for this sq chunk
                            for e in range(KD):
                                op = ctx_ps.tile([P, 512], f32, tag="ctx",
                                                 name=f"op_{e}_{c}")
                                nc.tensor.matmul(
                                    op[:],
                                    wo_a[:, e * P : (e + 1) * P],
                                    ctx_a[:, c * 512 : (c + 1) * 512],
                                    start=True, stop=False,
                                )
                                nc.tensor.matmul(
                                    op[:],
                                    wo_b[:, e * P : (e + 1) * P],
                                    ctx_b[:, c * 512 : (c + 1) * 512],
                                    start=False, stop=True,
                                )
                                o = work.tile([P, 512], f32, tag="o", bufs=6)
                                if e % 2 == 0:
                                    nc.vector.tensor_copy(o[:], op[:])
                                else:
                                    nc.scalar.activation(
                                        o[:], op[:],
                                        mybir.ActivationFunctionType.Copy,
                                    )
                                nc.sync.dma_start(
                                    out=outT_d[e][:, c * 512 : (c + 1) * 512], in_=o[:]
                                )
                            pe_fill(4)

    nc.finalize()
    return nc


def _get_nc(body_reps=1):
    key = ("nc", body_reps)
    if key not in _NC_CACHE:
        _NC_CACHE[key] = _build_bass(body_reps)
    return _NC_CACHE[key]


def _core_inputs(c, x, w_q, b_q, w_k, b_k, w_v, b_v, w_o):
    import ml_dtypes
    bf = ml_dtypes.bfloat16
    b, g = divmod(c, 4)
    gs = slice(g * E, (g + 1) * E)
    wv_pad = np.zeros((D, 256), bf)
    wv_pad[:, :E] = np.ascontiguousarray(w_v[gs, :].T).astype(bf)
    bv_pad = np.zeros((1, 256), bf)
    bv_pad[0, :E] = b_v[gs].astype(bf)
    return {
        "xT": np.ascontiguousarray(x[b].T).astype(bf),
        "wqT": np.ascontiguousarray(w_q[gs, :].T).astype(bf),
        "wkT": np.ascontiguousarray(w_k[gs, :].T).astype(bf),
        "wvT": wv_pad,
        "bq": b_q[gs].reshape(1, E).astype(bf),
        "bk": b_k[gs].reshape(1, E).astype(bf),
        "bv": bv_pad,
        "woT": np.ascontiguousarray(w_o[:, gs].T),
        "ones": np.ones((P, 512), np.float32),
    }


def kernel(x, w_q, b_q, w_k, b_k, w_v, b_v, w_o, b_o, _trace=False):
    from concourse.bass_utils import run_bass_kernel_spmd

    x = np.asarray(x, np.float32)
    args = [np.asarray(a, np.float32) for a in
            (w_q, b_q, w_k, b_k, w_v, b_v, w_o)]
    b_o = np.asarray(b_o, np.float32)

    nc = _get_nc()
    in_maps = [_core_inputs(c, x, *args) for c in range(8)]
    res = run_bass_kernel_spmd(nc, in_maps, core_ids=list(range(8)), trace=_trace)

    out = np.zeros((B, S, D), np.float32)
    for c in range(8):
        out[c // 4] += res.results[c]["outT"].T
    out += b_o
    if _trace:
        kernel._last_results = res
    return out
